# revision 38
# baseline (speedup 1.0000x reference)
"""Trainium2 Bass kernel for nn_MultiHeadAttention_88536455840315.

Math notes (vs the jax reference):
  - The second einsum (log_probs[..., None] * attn) @ v factors to
    log_probs[..., None] * (attn @ v) because log_probs does not depend on
    the key index.  So only two big attention matmuls are needed.
  - Softmax is computed without max subtraction: dots ~ N(0,1) here, so
    exp(dots*scale) never overflows fp32.
  - sumexp is fused into the attn@v matmul as a ones column appended to V.

Sharding (8 cores): core c handles batch c//4 and query rows
(c%4)*512 .. +512 of that batch.  Each core computes the full K/V for its
batch (replicated within the 4-core group; cross-core collectives are
either unsimulable -- remote_dma sem waits deadlock the single-core
timeline sim -- or cost-prohibitive via collective_compute).  The
per-core query offset is realized by rolling the batch rows host-side so
that each core's queries are always rows 0:512 (softmax is
permutation-invariant over keys, so rolling K/V order is exact).

Performance structure (vs the 238us baseline):
  - attn@V runs in the [query, dim] orientation: the exp tiles (bf16) are
    the PE stationary operand and V+ones (bf16) is the short (65-col)
    moving operand.  The cost model charges moving-columns only, so this
    halves attn@V PE time and yields the product directly in [q, d]
    layout -- no numerator copies or transposes.  bf16 exp/V adds ~1.7e-3
    rel err (validated against the fp64 reference).
  - V projection is split: heads 0-7 (psA) interleave into the
    x-load/transpose phase (block nb of V needs only column-block nb of
    x^T) and must finish before attn_v(0); heads 8-11 (psB) are only
    needed by attn_v(8) and spread over heads 1-8 as PE filler.
  - attn_v lags dots/exp by 1-2 heads (22-buffer bf16 exp ring), and
    all PE filler work (attn_v of earlier heads, K^T/Q^T chunk
    projections, deferred V-projection pieces) is emitted in small units
    INTERLEAVED between the dots tiles of each head: the dps
    double-buffer makes dots exp-paced, and the in-order PE queue would
    otherwise stall all work queued behind a waiting dots matmul.
  - ACT runs (almost) nothing but the softmax exp (the per-core floor,
    ~100us incl per-instruction access overhead); PSUM drains go to
    DVE (Pool cannot access PSUM on HW); SBUF-only accumulations go to
    Pool.  Weight loads are single strided DMAs issued from SP/ACT
    (hwdge), never gpsimd (software desc-gen burns Pool engine time).
  - Tail: mean/var come from precomputed partial sums (head 11 skips the
    ACC pass), the per-qt log-prob chains alternate DVE/Pool, OH^T
    transposes drain via ACT, and the bias is folded into the output
    matmul as a rank-1 ones x bias product so y streams straight from
    fin tiles.
"""

import sys

if "/opt/trn_rl_repo" not in sys.path:
    sys.path.insert(0, "/opt/trn_rl_repo")

import numpy as np

import concourse.bass as bass
import concourse.mybir as mybir
import concourse.tile as tile
from concourse import bacc
from concourse import bass_utils
from concourse.masks import make_identity

F32 = mybir.dt.float32
F32R = mybir.dt.float32r
BF16 = mybir.dt.bfloat16
AF = mybir.ActivationFunctionType
ALU = mybir.AluOpType
AX = mybir.AxisListType

B, N, E = 2, 2048, 768
H, DH = 12, 64
HD = H * DH            # 768
NQ = 512               # query rows per core
SCALE = DH ** -0.5
LOG2PI = float(np.log(2.0 * np.pi))
CONST = -0.5 * DH * LOG2PI   # -32*log(2*pi)

NE = E // 128          # 6 chunks of the embedding dim
NN = N // 128          # 16 chunks of the sequence
NQT = NQ // 128        # 4 query tiles
D1 = DH + 1            # head dim + sumexp column


def _emit(tc):
    nc = tc.nc
    xb = nc.dram_tensor("xb", [N, E], F32R, kind="ExternalInput").ap()
    wqkv = nc.dram_tensor("wqkv", [E, 3 * HD], F32R, kind="ExternalInput")
    wqkv_ap = wqkv.ap()
    wout = nc.dram_tensor("wout", [HD, E], F32R, kind="ExternalInput").ap()
    bout_t = nc.dram_tensor("bout", [E], F32R, kind="ExternalInput")
    y = nc.dram_tensor("y", [NQ, E], F32, kind="ExternalOutput").ap()

    with tc.tile_pool(name="consts", bufs=1) as consts, \
         tc.tile_pool(name="big", bufs=1) as big, \
         tc.tile_pool(name="wop", bufs=1) as wop:
        ident = consts.tile([128, 128], F32, name="ident", tag="ident")
        make_identity(nc, ident)
        ident_r = consts.tile([128, 128], F32R, name="identr", tag="identr")
        nc.vector.tensor_copy(ident_r, ident)
        ones_f = consts.tile([1, 128], F32, name="onesf", tag="onesf")
        nc.gpsimd.memset(ones_f, 1.0)
        ones_r = consts.tile([1, 128], F32R, name="ones", tag="ones")
        nc.vector.tensor_copy(ones_r, ones_f)

        # persistent SBUF tensors
        XT = [big.tile([128, N], F32R, name=f"xt{i}", tag=f"xt{i}")
              for i in range(NE)]
        VA = [big.tile([128, H, D1], BF16, name=f"va{j}", tag=f"va{j}")
              for j in range(NN)]
        PROD = big.tile([128, NQT, H, DH], F32, name="prod", tag="prod")
        ACCS = big.tile([128, NQT, DH], F32, name="accs", tag="accs")
        ACCQ = big.tile([128, NQT, DH], F32, name="accq", tag="accq")
        MA = big.tile([128, NQT, DH], F32, name="ma", tag="ma")
        QA = big.tile([128, NQT, DH], F32, name="qa", tag="qa")
        bias = big.tile([1, E], F32R, name="bias", tag="bias")

        # ones column for the fused sumexp
        for va in VA:
            nc.gpsimd.memset(va[:, :, DH:D1], 1.0)

        cp = 0  # copy-engine rotation counter

        with tc.tile_pool(name="jps", bufs=2, space="PSUM") as jps, \
             tc.tile_pool(name="wvp", bufs=1) as wvp, \
             tc.tile_pool(name="wqk", bufs=2) as wqk, \
             tc.tile_pool(name="ktp", bufs=2) as ktp, \
             tc.tile_pool(name="qtp", bufs=2) as qtp:

            # ---------------- K^T / Q^T projection helpers ----------------
            KT = {}
            QT = {}
            WQK0 = []

            def load_wqk(kc):
                """W_q and W_k column chunk kc as [128, 6, 128] tiles."""
                tq = wqk.tile([128, NE, 128], F32R, name="wq6", tag="wq6")
                tk = wqk.tile([128, NE, 128], F32R, name="wk6", tag="wk6")
                for t, col0 in ((tq, kc * 128), (tk, HD + kc * 128)):
                    nc.sync.dma_start(out=t, in_=bass.AP(
                        tensor=wqkv, offset=col0,
                        ap=[[3 * HD, 128], [128 * 3 * HD, NE], [1, 128]]))
                return tq, tk

            # ---------------- phase 1: x load + x^T + V projection --------
            WV = []

            def v_proj_a(nb, act_ok=False):
                """V heads 0-7 for block nb (needed by first attn_v)."""
                psA = jps.tile([128, 512], F32, name="vpa", tag="jp")
                for e in range(NE):
                    nc.tensor.matmul(
                        psA, XT[e][:, nb * 128:(nb + 1) * 128], WVA[e],
                        start=(e == 0), stop=(e == NE - 1))
                dst = VA[nb][:, 0:8, 0:DH]
                if act_ok:
                    nc.scalar.copy(dst, psA.rearrange("p (h d) -> p h d",
                                                      h=8))
                else:
                    nc.vector.tensor_copy(
                        dst, psA.rearrange("p (h d) -> p h d", h=8))

            def v_proj_b(nb):
                """V heads 8-11 for block nb (needed by attn_v(8))."""
                psB = jps.tile([128, 512], F32, name="vpb", tag="jp")
                for e in range(NE):
                    nc.tensor.matmul(
                        psB[:, 0:256], XT[e][:, nb * 128:(nb + 1) * 128],
                        WVB[e],
                        start=(e == 0), stop=(e == NE - 1))
                nc.vector.tensor_copy(
                    VA[nb][:, 8:12, 0:DH],
                    psB[:, 0:256].rearrange("p (h d) -> p h d", h=4))

            WVA = [wvp.tile([128, 512], F32R, name=f"wva{e}",
                           tag=f"wva{e}") for e in range(NE)]
            WVB = [wvp.tile([128, 256], F32R, name=f"wvb{e}",
                           tag=f"wvb{e}") for e in range(NE)]
            with tc.tile_pool(name="xld", bufs=8) as xld, \
                 tc.tile_pool(name="xps", bufs=2, space="PSUM") as xps:
                # DMA order: x group 0, then WV, then x groups 1..3.
                # V projection of group g is emitted after the transposes of
                # group g+1 so PE never waits on the WV arrival.
                for nbg in range(NN // 4):
                    if nbg == 3:
                        WQK0.append(load_wqk(0))
                    xins = []
                    for k in range(4):
                        nb = nbg * 4 + k
                        xin = xld.tile([128, E], F32R, name="xin", tag="xin")
                        deng = nc.scalar if k % 2 else nc.sync
                        deng.dma_start(
                            out=xin, in_=xb[nb * 128:(nb + 1) * 128, :])
                        xins.append(xin)
                    if nbg == 0:
                        for e in range(NE):
                            nc.sync.dma_start(
                                out=WVA[e],
                                in_=wqkv_ap[e * 128:(e + 1) * 128,
                                            2 * HD:2 * HD + 512])
                    for e in range(NE):
                        pt = xps.tile([128, 512], F32R, name="tp", tag="xp")
                        for k in range(4):
                            nc.tensor.transpose(
                                pt[:, k * 128:(k + 1) * 128],
                                xins[k][:, e * 128:(e + 1) * 128],
                                ident_r)
                        dst = XT[e][:, nbg * 512:(nbg + 1) * 512]
                        if cp % 2:
                            nc.scalar.copy(dst, pt)
                        else:
                            nc.vector.tensor_copy(dst, pt)
                        cp += 1
                    if nbg >= 1:
                        for k in range(4):
                            v_proj_a((nbg - 1) * 4 + k, act_ok=(nbg == 1))
            def qk_proj_units(kc, tq, tk):
                """Closures: Q^T piece then 4 K^T pieces for chunk kc."""
                kt = ktp.tile([128, N], F32R, name="kt", tag="kt")
                qt = qtp.tile([128, NQ], F32R, name="qt", tag="qt")
                KT[kc] = kt
                QT[kc] = qt

                def q_piece():
                    ps = jps.tile([128, 512], F32, name="qp", tag="jp")
                    for e in range(NE):
                        nc.tensor.matmul(ps, tq[:, e, :], XT[e][:, 0:NQ],
                                         start=(e == 0), stop=(e == NE - 1))
                    nc.vector.tensor_copy(qt, ps)

                def k_piece(nb):
                    ps = jps.tile([128, 512], F32, name="kp", tag="jp")
                    for e in range(NE):
                        nc.tensor.matmul(
                            ps, tk[:, e, :],
                            XT[e][:, nb * 512:(nb + 1) * 512],
                            start=(e == 0), stop=(e == NE - 1))
                    nc.vector.tensor_copy(kt[:, nb * 512:(nb + 1) * 512],
                                          ps)
                return [q_piece] + [
                    (lambda nb=nb: k_piece(nb)) for nb in range(4)]

            def qk_proj(kc, tq, tk):
                for u in qk_proj_units(kc, tq, tk):
                    u()

            WO = []

            # ---------------- attention main loop -------------------------
            with tc.tile_pool(name="dps", bufs=2, space="PSUM") as dps, \
                 tc.tile_pool(name="pps", bufs=2, space="PSUM") as pps, \
                 tc.tile_pool(name="expp", bufs=21) as expp:

                qk_proj(0, *WQK0[0])

                EXP = {}   # h -> list of 8 exp tiles
                PPS = {}   # h -> product psum tile

                def dots_exp(h, fillers=()):
                    """dots+exp for head h with PE filler units emitted
                    between dots tiles (the dps double-buffer makes dots
                    exp-paced; interleaved filler keeps PE busy)."""
                    fillers = list(fillers)
                    kc, pofs = h // 2, (h % 2) * DH
                    kt, qt = KT[kc], QT[kc]
                    qth = qt[pofs:pofs + DH, :]
                    exs = []
                    for jj in range(8):
                        dt_ = dps.tile([128, 2, NQ], F32, name="dots",
                                       tag="dots")
                        for k in range(2):
                            jb = jj * 2 + k
                            nc.tensor.matmul(
                                dt_[:, k, :],
                                kt[pofs:pofs + DH, jb * 128:(jb + 1) * 128],
                                qth, start=True, stop=True)
                        ex = expp.tile([128, 2, NQ], BF16, name="expd",
                                       tag="expd")
                        nc.scalar.activation(out=ex, in_=dt_, func=AF.Exp,
                                             scale=SCALE)
                        exs.append(ex)
                        if jj >= 1 and fillers:
                            fillers.pop(0)()
                    EXP[h] = exs
                    for f in fillers:
                        f()

                def attn_v_qt(h, qt_i):
                    """attn@V for head h, one query tile."""
                    exs = EXP[h]
                    if qt_i == 0:
                        PPS[h] = pps.tile([128, NQT, D1], F32, name="pp",
                                          tag="pp")
                    pp = PPS[h]
                    for jb in range(NN):
                        ex = exs[jb // 2]
                        st = ex[:, jb % 2, qt_i * 128:(qt_i + 1) * 128]
                        nc.tensor.matmul(
                            pp[:, qt_i, :], st, VA[jb][:, h, :],
                            start=(jb == 0), stop=(jb == NN - 1))
                    if qt_i == NQT - 1:
                        EXP.pop(h)

                def attn_v(h):
                    for qt_i in range(NQT):
                        attn_v_qt(h, qt_i)

                def normalize(h):
                    acc_eng = nc.gpsimd
                    pp = PPS.pop(h)
                    rsh = big.tile([128, NQT], F32, name="rsh", tag="rsh",
                                   bufs=3)
                    nc.vector.reciprocal(rsh, pp[:, :, DH:D1])
                    pvh = bass.AP(tensor=PROD.tensor,
                                  offset=PROD.offset + h * DH,
                                  ap=[PROD.ap[0], [H * DH, NQT], [1, DH]])
                    rsh_bc = bass.AP(tensor=rsh.tensor, offset=rsh.offset,
                                     ap=[rsh.ap[0], [1, NQT], [0, DH]])
                    nc.vector.tensor_tensor(out=pvh, in0=pp[:, :, 0:DH],
                                            in1=rsh_bc, op=ALU.mult)
                    if h == H - 1:
                        return
                    if h == 0:
                        acc_eng.tensor_copy(ACCS, pvh)
                        acc_eng.tensor_tensor(out=ACCQ, in0=pvh, in1=pvh,
                                              op=ALU.mult)
                    else:
                        sqh = big.tile([128, NQT, DH], F32, name="sqh",
                                       tag="sqh", bufs=2)
                        acc_eng.tensor_tensor(out=sqh, in0=pvh, in1=pvh,
                                              op=ALU.mult)
                        acc_eng.tensor_tensor(out=ACCS, in0=ACCS, in1=pvh,
                                              op=ALU.add)
                        acc_eng.tensor_tensor(out=ACCQ, in0=ACCQ, in1=sqh,
                                              op=ALU.add)

                done_av = 0

                def av_units(h):
                    """attn_v for head h as 4 filler units; normalize
                    rides with the last qt."""
                    us = [(lambda q=q: attn_v_qt(h, q))
                          for q in range(NQT - 1)]

                    def last():
                        attn_v_qt(h, NQT - 1)
                        normalize(h)
                    return us + [last]

                def drain_units(upto):
                    nonlocal done_av
                    us = []
                    while done_av < upto:
                        us += av_units(done_av)
                        done_av += 1
                    return us

                for h in range(H):
                    fillers = []
                    if h % 2 == 1 and h < H - 1:
                        tq, tk = load_wqk(h // 2 + 1)
                        fillers += qk_proj_units(h // 2 + 1, tq, tk)
                    if h == 0:
                        for e in range(NE):
                            nc.sync.dma_start(
                                out=WVB[e],
                                in_=wqkv_ap[e * 128:(e + 1) * 128,
                                            2 * HD + 512:3 * HD])
                        fillers += [(lambda k=k: v_proj_a(12 + k))
                                    for k in range(4)]
                    if h == 2:
                        fillers = drain_units(2) + fillers
                    elif h >= 3:
                        upto = {9: 8, 10: 9, 11: 11}.get(h, h)
                        fillers = drain_units(upto) + fillers
                    if 2 <= h <= 9:
                        fillers += [(lambda k=k: v_proj_b(2 * (h - 2) + k))
                                    for k in range(2)]
                    if h == 8:
                        # W_out + bias loads, overlapped with late attention
                        nc.sync.dma_start(out=bias, in_=bass.AP(
                            tensor=bout_t, offset=0, ap=[[0, 1], [1, E]]))
                        for c in range(NE):
                            t = wop.tile([128, E], F32R, name=f"wo{c}",
                                         tag=f"wo{c}")
                            nc.sync.dma_start(
                                out=t, in_=wout[c * 128:(c + 1) * 128, :])
                            WO.append(t)
                    if h == H - 1:
                        def ma_qa():
                            nc.vector.tensor_scalar_mul(MA, ACCS, 1.0 / H)
                            nc.vector.tensor_scalar_mul(QA, ACCQ,
                                                        1.0 / (H - 1))
                        fillers.append(ma_qa)
                    dots_exp(h, fillers)
                for u in drain_units(H - 1):
                    u()
                for q in range(NQT):
                    attn_v_qt(H - 1, q)
                pp11 = PPS.pop(H - 1)
                for q in range(NQT):
                    rs = big.tile([128, 1], F32, name="rs11", tag="rs11",
                                  bufs=4)
                    nc.vector.reciprocal(rs, pp11[:, q, DH:D1])
                    pvh = bass.AP(
                        tensor=PROD.tensor,
                        offset=PROD.offset + q * H * DH + (H - 1) * DH,
                        ap=[PROD.ap[0], [1, DH]])
                    rs_bc = bass.AP(tensor=rs.tensor, offset=rs.offset,
                                    ap=[rs.ap[0], [0, DH]])
                    nc.vector.tensor_tensor(out=pvh, in0=pp11[:, q, 0:DH],
                                            in1=rs_bc, op=ALU.mult)
                    maq = MA[:, q, :]
                    nc.vector.scalar_tensor_tensor(
                        out=maq, in0=pvh, scalar=1.0 / H, in1=maq,
                        op0=ALU.mult, op1=ALU.add)
                    sqt = big.tile([128, DH], F32, name="sqt", tag="sqt",
                                   bufs=2)
                    nc.gpsimd.tensor_tensor(out=sqt, in0=pvh, in1=pvh,
                                            op=ALU.mult)
                    qaq = QA[:, q, :]
                    nc.vector.scalar_tensor_tensor(
                        out=qaq, in0=sqt, scalar=1.0 / (H - 1), in1=qaq,
                        op0=ALU.mult, op1=ALU.add)
                    m2t = big.tile([128, DH], F32, name="m2t", tag="m2t",
                                   bufs=2)
                    nc.vector.scalar_tensor_tensor(
                        out=m2t, in0=maq, scalar=H / (H - 1.0), in1=maq,
                        op0=ALU.mult, op1=ALU.mult)
                    nc.gpsimd.tensor_tensor(out=qaq, in0=qaq, in1=m2t,
                                            op=ALU.subtract)

        # ---------------- statistics / log-prob weighting ----------------
        # Batched all-qt preamble on DVE, then a per-qt chain (diff on DVE,
        # square on ACT, mult+reduce on DVE, OH on Pool) pipelined into the
        # PE transposes / output projection of earlier qts.
        with tc.tile_pool(name="ohp", bufs=1) as ohp, \
             tc.tile_pool(name="wkp", bufs=1) as wkp, \
             tc.tile_pool(name="tp2", bufs=2, space="PSUM") as tp2p, \
             tc.tile_pool(name="fps", bufs=3, space="PSUM") as fps, \
             tc.tile_pool(name="ohtp", bufs=1) as ohtp, \
             tc.tile_pool(name="finp", bufs=2) as finp:
            mean, var = MA, QA
            OHT = [ohtp.tile([128, NQ], F32R, name=f"oht{c}", tag=f"oht{c}")
                   for c in range(NE)]
            for qt_i in range(NQT):
                pvq = bass.AP(tensor=PROD.tensor,
                              offset=PROD.offset + qt_i * H * DH,
                              ap=[PROD.ap[0], [DH, H], [1, DH]])
                varq = var[:, qt_i, :]
                rvar = wkp.tile([128, DH], F32, name="rvar", tag="rvar",
                                bufs=2)
                nc.vector.reciprocal(rvar, varq)
                lv = wkp.tile([128, DH], F32, name="lv", tag="lv", bufs=2)
                S = wkp.tile([128, 1], F32, name="S", tag="S", bufs=2)
                nc.scalar.activation(out=lv, in_=varq, func=AF.Ln,
                                     accum_out=S)
                cs = wkp.tile([128, 1], F32, name="cs", tag="cs", bufs=2)
                nc.vector.tensor_scalar(out=cs, in0=S, scalar1=-1.0,
                                        scalar2=CONST, op0=ALU.mult,
                                        op1=ALU.add)
                diff = wkp.tile([128, H, DH], F32, name="diff", tag="diff",
                                bufs=2)
                mean_bc = bass.AP(tensor=mean.tensor,
                                  offset=mean.offset + qt_i * DH,
                                  ap=[mean.ap[0], [0, H], [1, DH]])
                sq_eng = nc.gpsimd if qt_i in (1, 2) else nc.vector
                nc.vector.tensor_tensor(out=diff, in0=pvq, in1=mean_bc,
                                        op=ALU.subtract)
                sq_eng.tensor_tensor(out=diff, in0=diff, in1=diff,
                                     op=ALU.mult)
                rvar_bc = bass.AP(tensor=rvar.tensor,
                                  offset=rvar.offset,
                                  ap=[rvar.ap[0], [0, H], [1, DH]])
                sq_eng.tensor_tensor(out=diff, in0=diff, in1=rvar_bc,
                                     op=ALU.mult)
                lp0 = wkp.tile([128, H], F32, name="lp0", tag="lp0", bufs=2)
                nc.vector.reduce_sum(lp0, diff, axis=AX.X)
                lp = wkp.tile([128, H], F32, name="lp", tag="lp", bufs=2)
                nc.vector.tensor_scalar(out=lp, in0=lp0, scalar1=0.25,
                                        scalar2=cs,
                                        op0=ALU.mult, op1=ALU.add)
                oh = ohp.tile([128, H * DH], F32R, name="oh", tag="oh",
                              bufs=2)
                ohv = oh.rearrange("p (h d) -> p h d", h=H)
                lp_bc = bass.AP(tensor=lp.tensor, offset=lp.offset,
                                ap=[lp.ap[0], [1, H], [0, DH]])
                oh_eng = nc.vector if qt_i in (0, 3) else nc.gpsimd
                oh_eng.tensor_tensor(out=ohv, in0=pvq, in1=lp_bc,
                                     op=ALU.mult)
                # transpose OH block-wise into OHT, then project
                for c in range(NE):
                    tp = tp2p.tile([128, 128], F32R, name="t2", tag="t2")
                    nc.tensor.transpose(
                        tp, oh[:, c * 128:(c + 1) * 128], ident_r)
                    nc.scalar.copy(
                        OHT[c][:, qt_i * 128:(qt_i + 1) * 128], tp)
                psA = fps.tile([128, 512], F32, name="fA", tag="f")
                psB = fps.tile([128, 256], F32, name="fB", tag="f")
                nc.tensor.matmul(psA, ones_r, bias[:, 0:512],
                                 start=True, stop=False)
                nc.tensor.matmul(psB, ones_r, bias[:, 512:768],
                                 start=True, stop=False)
                for c in range(NE):
                    nc.tensor.matmul(psA,
                                     OHT[c][:, qt_i * 128:(qt_i + 1) * 128],
                                     WO[c][:, 0:512],
                                     start=False, stop=(c == NE - 1))
                for c in range(NE):
                    nc.tensor.matmul(psB,
                                     OHT[c][:, qt_i * 128:(qt_i + 1) * 128],
                                     WO[c][:, 512:768],
                                     start=False, stop=(c == NE - 1))
                fin = finp.tile([128, E], F32, name="fin", tag="fin")
                nc.scalar.copy(fin[:, 0:512], psA)
                nc.sync.dma_start(out=y[qt_i * 128:(qt_i + 1) * 128, 0:512],
                                  in_=fin[:, 0:512])
                nc.scalar.copy(fin[:, 512:768], psB)
                nc.sync.dma_start(out=y[qt_i * 128:(qt_i + 1) * 128, 512:768],
                                  in_=fin[:, 512:768])


_NC_CACHE = {}


def _get_nc():
    if "nc" not in _NC_CACHE:
        nc = bacc.Bacc("TRN2", target_bir_lowering=False, debug=False,
                       num_devices=8)
        with tile.TileContext(nc) as tc:
            _emit(tc)
        nc.compile()
        _NC_CACHE["nc"] = nc
    return _NC_CACHE["nc"]


def kernel(x, w_qkv, w_out, b_out):
    x = np.ascontiguousarray(x, dtype=np.float32)
    w_qkv = np.ascontiguousarray(w_qkv, dtype=np.float32)
    w_out = np.ascontiguousarray(w_out, dtype=np.float32)
    b_out = np.ascontiguousarray(b_out, dtype=np.float32)
    assert x.shape == (B, N, E)

    nc = _get_nc()
    in_maps = []
    for c in range(8):
        beta, qoff = c // 4, (c % 4) * NQ
        xbc = np.ascontiguousarray(np.roll(x[beta], -qoff, axis=0))
        in_maps.append({"xb": xbc, "wqkv": w_qkv, "wout": w_out,
                        "bout": b_out})
    res = bass_utils.run_bass_kernel_spmd(nc, in_maps, core_ids=list(range(8)))
    out = np.empty((B, N, E), dtype=np.float32)
    for c in range(8):
        beta, qoff = c // 4, (c % 4) * NQ
        out[beta, qoff:qoff + NQ, :] = res.results[c]["y"]
    return out


# revision 39
# speedup vs baseline: 1.0155x; 1.0155x over previous
"""Trainium2 Bass kernel for nn_MultiHeadAttention_88536455840315.

Math notes (vs the jax reference):
  - The second einsum (log_probs[..., None] * attn) @ v factors to
    log_probs[..., None] * (attn @ v) because log_probs does not depend on
    the key index.  So only two big attention matmuls are needed.
  - Softmax is computed without max subtraction: dots ~ N(0,1) here, so
    exp(dots*scale) never overflows fp32.
  - sumexp is fused into the attn@v matmul as a ones column appended to V.

Sharding (8 cores): core c handles batch c//4 and query rows
(c%4)*512 .. +512 of that batch.  Each core computes the full K/V for its
batch (replicated within the 4-core group; cross-core collectives are
either unsimulable -- remote_dma sem waits deadlock the single-core
timeline sim -- or cost-prohibitive via collective_compute).  The
per-core query offset is realized by rolling the batch rows host-side so
that each core's queries are always rows 0:512 (softmax is
permutation-invariant over keys, so rolling K/V order is exact).

Performance structure (vs the 238us baseline):
  - attn@V runs in the [query, dim] orientation: the exp tiles (bf16) are
    the PE stationary operand and V+ones (bf16) is the short (65-col)
    moving operand.  The cost model charges moving-columns only, so this
    halves attn@V PE time and yields the product directly in [q, d]
    layout -- no numerator copies or transposes.  bf16 exp/V adds ~1.7e-3
    rel err (validated against the fp64 reference).
  - V projection is split: heads 0-7 (psA) interleave into the
    x-load/transpose phase (block nb of V needs only column-block nb of
    x^T) and must finish before attn_v(0); heads 8-11 (psB) are only
    needed by attn_v(8) and spread over heads 1-8 as PE filler.
  - attn_v lags dots/exp by 1-2 heads (22-buffer bf16 exp ring), and
    all PE filler work (attn_v of earlier heads, K^T/Q^T chunk
    projections, deferred V-projection pieces) is emitted in small units
    INTERLEAVED between the dots tiles of each head: the dps
    double-buffer makes dots exp-paced, and the in-order PE queue would
    otherwise stall all work queued behind a waiting dots matmul.
  - ACT runs (almost) nothing but the softmax exp (the per-core floor,
    ~100us incl per-instruction access overhead); PSUM drains go to
    DVE (Pool cannot access PSUM on HW); SBUF-only accumulations go to
    Pool.  Weight loads are single strided DMAs issued from SP/ACT
    (hwdge), never gpsimd (software desc-gen burns Pool engine time).
  - Tail: mean/var come from precomputed partial sums (head 11 skips the
    ACC pass), the per-qt log-prob chains alternate DVE/Pool, OH^T
    transposes drain via ACT, and the bias is folded into the output
    matmul as a rank-1 ones x bias product so y streams straight from
    fin tiles.
"""

import sys

if "/opt/trn_rl_repo" not in sys.path:
    sys.path.insert(0, "/opt/trn_rl_repo")

import numpy as np

import concourse.bass as bass
import concourse.mybir as mybir
import concourse.tile as tile
from concourse import bacc
from concourse import bass_utils
from concourse.masks import make_identity

F32 = mybir.dt.float32
F32R = mybir.dt.float32r
BF16 = mybir.dt.bfloat16
AF = mybir.ActivationFunctionType
ALU = mybir.AluOpType
AX = mybir.AxisListType

B, N, E = 2, 2048, 768
H, DH = 12, 64
HD = H * DH            # 768
NQ = 512               # query rows per core
SCALE = DH ** -0.5
LOG2PI = float(np.log(2.0 * np.pi))
CONST = -0.5 * DH * LOG2PI   # -32*log(2*pi)

NE = E // 128          # 6 chunks of the embedding dim
NN = N // 128          # 16 chunks of the sequence
NQT = NQ // 128        # 4 query tiles
D1 = DH + 1            # head dim + sumexp column


def _emit(tc):
    nc = tc.nc
    xb = nc.dram_tensor("xb", [N, E], F32R, kind="ExternalInput").ap()
    wqkv = nc.dram_tensor("wqkv", [E, 3 * HD], F32R, kind="ExternalInput")
    wqkv_ap = wqkv.ap()
    wout = nc.dram_tensor("wout", [HD, E], F32R, kind="ExternalInput").ap()
    bout_t = nc.dram_tensor("bout", [E], F32R, kind="ExternalInput")
    y = nc.dram_tensor("y", [NQ, E], F32, kind="ExternalOutput").ap()

    with tc.tile_pool(name="consts", bufs=1) as consts, \
         tc.tile_pool(name="big", bufs=1) as big, \
         tc.tile_pool(name="wop", bufs=1) as wop:
        ident = consts.tile([128, 128], F32, name="ident", tag="ident")
        make_identity(nc, ident)
        ident_r = consts.tile([128, 128], F32R, name="identr", tag="identr")
        nc.vector.tensor_copy(ident_r, ident)
        ones_f = consts.tile([1, 128], F32, name="onesf", tag="onesf")
        nc.gpsimd.memset(ones_f, 1.0)
        ones_r = consts.tile([1, 128], F32R, name="ones", tag="ones")
        nc.vector.tensor_copy(ones_r, ones_f)

        # persistent SBUF tensors
        XT = [big.tile([128, N], F32R, name=f"xt{i}", tag=f"xt{i}")
              for i in range(NE)]
        VA = [big.tile([128, H, D1], BF16, name=f"va{j}", tag=f"va{j}")
              for j in range(NN)]
        PROD = big.tile([128, NQT, H, DH], F32, name="prod", tag="prod")
        ACCS = big.tile([128, NQT, DH], F32, name="accs", tag="accs")
        ACCQ = big.tile([128, NQT, DH], F32, name="accq", tag="accq")
        MA = big.tile([128, NQT, DH], F32, name="ma", tag="ma")
        QA = big.tile([128, NQT, DH], F32, name="qa", tag="qa")
        bias = big.tile([1, E], F32R, name="bias", tag="bias")

        # ones column for the fused sumexp
        for va in VA:
            nc.gpsimd.memset(va[:, :, DH:D1], 1.0)

        cp = 0  # copy-engine rotation counter

        with tc.tile_pool(name="jps", bufs=2, space="PSUM") as jps, \
             tc.tile_pool(name="wvp", bufs=1) as wvp, \
             tc.tile_pool(name="wqk", bufs=2) as wqk, \
             tc.tile_pool(name="ktp", bufs=2) as ktp, \
             tc.tile_pool(name="qtp", bufs=2) as qtp:

            # ---------------- K^T / Q^T projection helpers ----------------
            KT = {}
            QT = {}
            WQK0 = []

            def load_wqk(kc):
                """W_q and W_k column chunk kc as [128, 6, 128] tiles."""
                tq = wqk.tile([128, NE, 128], F32R, name="wq6", tag="wq6")
                tk = wqk.tile([128, NE, 128], F32R, name="wk6", tag="wk6")
                for t, col0 in ((tq, kc * 128), (tk, HD + kc * 128)):
                    nc.sync.dma_start(out=t, in_=bass.AP(
                        tensor=wqkv, offset=col0,
                        ap=[[3 * HD, 128], [128 * 3 * HD, NE], [1, 128]]))
                return tq, tk

            # ---------------- phase 1: x load + x^T + V projection --------
            WV = []

            def v_proj_a(nb, act_ok=False):
                """V heads 0-7 for block nb (needed by first attn_v)."""
                psA = jps.tile([128, 512], F32, name="vpa", tag="jp")
                for e in range(NE):
                    nc.tensor.matmul(
                        psA, XT[e][:, nb * 128:(nb + 1) * 128], WVA[e],
                        start=(e == 0), stop=(e == NE - 1))
                dst = VA[nb][:, 0:8, 0:DH]
                if act_ok:
                    nc.scalar.copy(dst, psA.rearrange("p (h d) -> p h d",
                                                      h=8))
                else:
                    nc.vector.tensor_copy(
                        dst, psA.rearrange("p (h d) -> p h d", h=8))

            def v_proj_b(nb):
                """V heads 8-11 for block nb (needed by attn_v(8))."""
                psB = jps.tile([128, 512], F32, name="vpb", tag="jp")
                for e in range(NE):
                    nc.tensor.matmul(
                        psB[:, 0:256], XT[e][:, nb * 128:(nb + 1) * 128],
                        WVB[e],
                        start=(e == 0), stop=(e == NE - 1))
                nc.vector.tensor_copy(
                    VA[nb][:, 8:12, 0:DH],
                    psB[:, 0:256].rearrange("p (h d) -> p h d", h=4))

            WVA = [wvp.tile([128, 512], F32R, name=f"wva{e}",
                           tag=f"wva{e}") for e in range(NE)]
            WVB = [wvp.tile([128, 256], F32R, name=f"wvb{e}",
                           tag=f"wvb{e}") for e in range(NE)]
            with tc.tile_pool(name="xld", bufs=8) as xld, \
                 tc.tile_pool(name="xps", bufs=2, space="PSUM") as xps:
                # DMA order: x group 0, then WV, then x groups 1..3.
                # V projection of group g is emitted after the transposes of
                # group g+1 so PE never waits on the WV arrival.
                for nbg in range(NN // 4):
                    if nbg == 3:
                        WQK0.append(load_wqk(0))
                    xins = []
                    for k in range(4):
                        nb = nbg * 4 + k
                        xin = xld.tile([128, E], F32R, name="xin", tag="xin")
                        deng = nc.scalar if k % 2 else nc.sync
                        deng.dma_start(
                            out=xin, in_=xb[nb * 128:(nb + 1) * 128, :])
                        xins.append(xin)
                    if nbg == 0:
                        for e in range(NE):
                            nc.sync.dma_start(
                                out=WVA[e],
                                in_=wqkv_ap[e * 128:(e + 1) * 128,
                                            2 * HD:2 * HD + 512])
                    for e in range(NE):
                        pt = xps.tile([128, 512], F32R, name="tp", tag="xp")
                        for k in range(4):
                            nc.tensor.transpose(
                                pt[:, k * 128:(k + 1) * 128],
                                xins[k][:, e * 128:(e + 1) * 128],
                                ident_r)
                        dst = XT[e][:, nbg * 512:(nbg + 1) * 512]
                        if cp % 2:
                            nc.scalar.copy(dst, pt)
                        else:
                            nc.vector.tensor_copy(dst, pt)
                        cp += 1
                    if nbg >= 1:
                        for k in range(4):
                            v_proj_a((nbg - 1) * 4 + k, act_ok=(nbg == 1))
            def qk_proj_units(kc, tq, tk):
                """Closures: Q^T piece then 4 K^T pieces for chunk kc."""
                kt = ktp.tile([128, N], F32R, name="kt", tag="kt")
                qt = qtp.tile([128, NQ], F32R, name="qt", tag="qt")
                KT[kc] = kt
                QT[kc] = qt

                def q_piece():
                    ps = jps.tile([128, 512], F32, name="qp", tag="jp")
                    for e in range(NE):
                        nc.tensor.matmul(ps, tq[:, e, :], XT[e][:, 0:NQ],
                                         start=(e == 0), stop=(e == NE - 1))
                    nc.vector.tensor_copy(qt, ps)

                def k_piece(nb):
                    ps = jps.tile([128, 512], F32, name="kp", tag="jp")
                    for e in range(NE):
                        nc.tensor.matmul(
                            ps, tk[:, e, :],
                            XT[e][:, nb * 512:(nb + 1) * 512],
                            start=(e == 0), stop=(e == NE - 1))
                    nc.vector.tensor_copy(kt[:, nb * 512:(nb + 1) * 512],
                                          ps)
                return [q_piece] + [
                    (lambda nb=nb: k_piece(nb)) for nb in range(4)]

            def qk_proj(kc, tq, tk):
                for u in qk_proj_units(kc, tq, tk):
                    u()

            WO = []

            # ---------------- attention main loop -------------------------
            with tc.tile_pool(name="dps", bufs=2, space="PSUM") as dps, \
                 tc.tile_pool(name="pps", bufs=2, space="PSUM") as pps, \
                 tc.tile_pool(name="expp", bufs=22) as expp:

                qk_proj(0, *WQK0[0])

                EXP = {}   # h -> list of 8 exp tiles
                PPS = {}   # h -> product psum tile

                def dots_exp(h, fillers=()):
                    """dots+exp for head h with PE filler units emitted
                    between dots tiles (the dps double-buffer makes dots
                    exp-paced; interleaved filler keeps PE busy)."""
                    fillers = list(fillers)
                    kc, pofs = h // 2, (h % 2) * DH
                    kt, qt = KT[kc], QT[kc]
                    qth = qt[pofs:pofs + DH, :]
                    exs = []
                    for jj in range(8):
                        dt_ = dps.tile([128, 2, NQ], F32, name="dots",
                                       tag="dots")
                        for k in range(2):
                            jb = jj * 2 + k
                            nc.tensor.matmul(
                                dt_[:, k, :],
                                kt[pofs:pofs + DH, jb * 128:(jb + 1) * 128],
                                qth, start=True, stop=True)
                        ex = expp.tile([128, 2, NQ], BF16, name="expd",
                                       tag="expd")
                        nc.scalar.activation(out=ex, in_=dt_, func=AF.Exp,
                                             scale=SCALE)
                        exs.append(ex)
                        if jj >= 1 and fillers:
                            fillers.pop(0)()
                    EXP[h] = exs
                    for f in fillers:
                        f()

                def attn_v_qt(h, qt_i):
                    """attn@V for head h, one query tile."""
                    exs = EXP[h]
                    if qt_i == 0:
                        PPS[h] = pps.tile([128, NQT, D1], F32, name="pp",
                                          tag="pp")
                    pp = PPS[h]
                    for jb in range(NN):
                        ex = exs[jb // 2]
                        st = ex[:, jb % 2, qt_i * 128:(qt_i + 1) * 128]
                        nc.tensor.matmul(
                            pp[:, qt_i, :], st, VA[jb][:, h, :],
                            start=(jb == 0), stop=(jb == NN - 1))
                    if qt_i == NQT - 1:
                        EXP.pop(h)

                def attn_v(h):
                    for qt_i in range(NQT):
                        attn_v_qt(h, qt_i)

                def normalize(h):
                    acc_eng = nc.gpsimd
                    pp = PPS.pop(h)
                    rsh = big.tile([128, NQT], F32, name="rsh", tag="rsh",
                                   bufs=3)
                    nc.vector.reciprocal(rsh, pp[:, :, DH:D1])
                    pvh = bass.AP(tensor=PROD.tensor,
                                  offset=PROD.offset + h * DH,
                                  ap=[PROD.ap[0], [H * DH, NQT], [1, DH]])
                    rsh_bc = bass.AP(tensor=rsh.tensor, offset=rsh.offset,
                                     ap=[rsh.ap[0], [1, NQT], [0, DH]])
                    nc.vector.tensor_tensor(out=pvh, in0=pp[:, :, 0:DH],
                                            in1=rsh_bc, op=ALU.mult)
                    if h == H - 1:
                        return
                    if h == 0:
                        acc_eng.tensor_copy(ACCS, pvh)
                        acc_eng.tensor_tensor(out=ACCQ, in0=pvh, in1=pvh,
                                              op=ALU.mult)
                    else:
                        sqh = big.tile([128, NQT, DH], F32, name="sqh",
                                       tag="sqh", bufs=2)
                        acc_eng.tensor_tensor(out=sqh, in0=pvh, in1=pvh,
                                              op=ALU.mult)
                        acc_eng.tensor_tensor(out=ACCS, in0=ACCS, in1=pvh,
                                              op=ALU.add)
                        acc_eng.tensor_tensor(out=ACCQ, in0=ACCQ, in1=sqh,
                                              op=ALU.add)

                done_av = 0

                def av_units(h):
                    """attn_v for head h as 4 filler units; normalize
                    rides with the last qt."""
                    us = [(lambda q=q: attn_v_qt(h, q))
                          for q in range(NQT - 1)]

                    def last():
                        attn_v_qt(h, NQT - 1)
                        normalize(h)
                    return us + [last]

                def drain_units(upto):
                    nonlocal done_av
                    us = []
                    while done_av < upto:
                        us += av_units(done_av)
                        done_av += 1
                    return us

                for h in range(H):
                    fillers = []
                    if h % 2 == 1 and h < H - 1:
                        tq, tk = load_wqk(h // 2 + 1)
                        fillers += qk_proj_units(h // 2 + 1, tq, tk)
                    if h == 0:
                        for e in range(NE):
                            nc.sync.dma_start(
                                out=WVB[e],
                                in_=wqkv_ap[e * 128:(e + 1) * 128,
                                            2 * HD + 512:3 * HD])
                        fillers += [(lambda k=k: v_proj_a(12 + k))
                                    for k in range(4)]
                    if h == 2:
                        fillers = drain_units(2) + fillers
                    elif h >= 3:
                        upto = {9: 8, 10: 9, 11: 11}.get(h, h)
                        fillers = drain_units(upto) + fillers
                    if 2 <= h <= 9:
                        fillers += [(lambda k=k: v_proj_b(2 * (h - 2) + k))
                                    for k in range(2)]
                    if h == 8:
                        # W_out + bias loads, overlapped with late attention
                        nc.sync.dma_start(out=bias, in_=bass.AP(
                            tensor=bout_t, offset=0, ap=[[0, 1], [1, E]]))
                        for c in range(NE):
                            t = wop.tile([128, E], F32R, name=f"wo{c}",
                                         tag=f"wo{c}")
                            nc.sync.dma_start(
                                out=t, in_=wout[c * 128:(c + 1) * 128, :])
                            WO.append(t)
                    if h == H - 1:
                        def ma_qa():
                            nc.vector.tensor_scalar_mul(MA, ACCS, 1.0 / H)
                            nc.vector.tensor_scalar_mul(QA, ACCQ,
                                                        1.0 / (H - 1))
                        fillers.append(ma_qa)
                    dots_exp(h, fillers)
                for u in drain_units(H):
                    u()

        # ---------------- statistics / log-prob weighting ----------------
        # Batched all-qt preamble on DVE, then a per-qt chain (diff on DVE,
        # square on ACT, mult+reduce on DVE, OH on Pool) pipelined into the
        # PE transposes / output projection of earlier qts.
        with tc.tile_pool(name="ohp", bufs=1) as ohp, \
             tc.tile_pool(name="wkp", bufs=1) as wkp, \
             tc.tile_pool(name="tp2", bufs=2, space="PSUM") as tp2p, \
             tc.tile_pool(name="fps", bufs=3, space="PSUM") as fps, \
             tc.tile_pool(name="ohtp", bufs=1) as ohtp, \
             tc.tile_pool(name="finp", bufs=2) as finp:
            p11 = bass.AP(tensor=PROD.tensor,
                          offset=PROD.offset + (H - 1) * DH,
                          ap=[PROD.ap[0], [H * DH, NQT], [1, DH]])
            mean = wkp.tile([128, NQT, DH], F32, name="mean", tag="mean")
            nc.vector.scalar_tensor_tensor(out=mean, in0=p11,
                                           scalar=1.0 / H, in1=MA,
                                           op0=ALU.mult, op1=ALU.add)
            sq11 = wkp.tile([128, NQT, DH], F32, name="sq11", tag="sq11")
            nc.vector.tensor_tensor(out=sq11, in0=p11, in1=p11, op=ALU.mult)
            qv = wkp.tile([128, NQT, DH], F32, name="qv", tag="qv")
            nc.vector.scalar_tensor_tensor(out=qv, in0=sq11,
                                           scalar=1.0 / (H - 1), in1=QA,
                                           op0=ALU.mult, op1=ALU.add)
            # m2s = (H/(H-1)) * mean^2
            m2s = wkp.tile([128, NQT, DH], F32, name="m2s", tag="m2s")
            nc.vector.scalar_tensor_tensor(out=m2s, in0=mean,
                                           scalar=H / (H - 1.0), in1=mean,
                                           op0=ALU.mult, op1=ALU.mult)
            var = wkp.tile([128, NQT, DH], F32, name="var", tag="var")
            nc.vector.tensor_tensor(out=var, in0=qv, in1=m2s,
                                    op=ALU.subtract)
            rvar = wkp.tile([128, NQT, DH], F32, name="rvar", tag="rvar")
            nc.vector.reciprocal(rvar, var)
            lv = wkp.tile([128, NQT, DH], F32, name="lv", tag="lv")
            nc.scalar.activation(out=lv, in_=var, func=AF.Ln)
            S = wkp.tile([128, NQT], F32, name="S", tag="S")
            nc.vector.reduce_sum(S, lv, axis=AX.X)
            cs = wkp.tile([128, NQT], F32, name="cs", tag="cs")
            nc.vector.tensor_scalar(out=cs, in0=S, scalar1=-1.0,
                                    scalar2=CONST, op0=ALU.mult, op1=ALU.add)
            OHT = [ohtp.tile([128, NQ], F32R, name=f"oht{c}", tag=f"oht{c}")
                   for c in range(NE)]
            for qt_i in range(NQT):
                pvq = bass.AP(tensor=PROD.tensor,
                              offset=PROD.offset + qt_i * H * DH,
                              ap=[PROD.ap[0], [DH, H], [1, DH]])
                diff = wkp.tile([128, H, DH], F32, name="diff", tag="diff",
                                bufs=2)
                mean_bc = bass.AP(tensor=mean.tensor,
                                  offset=mean.offset + qt_i * DH,
                                  ap=[mean.ap[0], [0, H], [1, DH]])
                sq_eng = nc.gpsimd if qt_i in (1, 2) else nc.vector
                nc.vector.tensor_tensor(out=diff, in0=pvq, in1=mean_bc,
                                        op=ALU.subtract)
                sq_eng.tensor_tensor(out=diff, in0=diff, in1=diff,
                                     op=ALU.mult)
                rvar_bc = bass.AP(tensor=rvar.tensor,
                                  offset=rvar.offset + qt_i * DH,
                                  ap=[rvar.ap[0], [0, H], [1, DH]])
                sq_eng.tensor_tensor(out=diff, in0=diff, in1=rvar_bc,
                                     op=ALU.mult)
                lp0 = wkp.tile([128, H], F32, name="lp0", tag="lp0", bufs=2)
                nc.vector.reduce_sum(lp0, diff, axis=AX.X)
                lp = wkp.tile([128, H], F32, name="lp", tag="lp", bufs=2)
                nc.vector.tensor_scalar(out=lp, in0=lp0, scalar1=0.25,
                                        scalar2=cs[:, qt_i:qt_i + 1],
                                        op0=ALU.mult, op1=ALU.add)
                oh = ohp.tile([128, H * DH], F32R, name="oh", tag="oh",
                              bufs=2)
                ohv = oh.rearrange("p (h d) -> p h d", h=H)
                lp_bc = bass.AP(tensor=lp.tensor, offset=lp.offset,
                                ap=[lp.ap[0], [1, H], [0, DH]])
                oh_eng = nc.vector if qt_i in (0, 3) else nc.gpsimd
                oh_eng.tensor_tensor(out=ohv, in0=pvq, in1=lp_bc,
                                     op=ALU.mult)
                # transpose OH block-wise into OHT, then project
                for c in range(NE):
                    tp = tp2p.tile([128, 128], F32R, name="t2", tag="t2")
                    nc.tensor.transpose(
                        tp, oh[:, c * 128:(c + 1) * 128], ident_r)
                    nc.scalar.copy(
                        OHT[c][:, qt_i * 128:(qt_i + 1) * 128], tp)
                psA = fps.tile([128, 512], F32, name="fA", tag="f")
                psB = fps.tile([128, 256], F32, name="fB", tag="f")
                nc.tensor.matmul(psA, ones_r, bias[:, 0:512],
                                 start=True, stop=False)
                nc.tensor.matmul(psB, ones_r, bias[:, 512:768],
                                 start=True, stop=False)
                for c in range(NE):
                    nc.tensor.matmul(psA,
                                     OHT[c][:, qt_i * 128:(qt_i + 1) * 128],
                                     WO[c][:, 0:512],
                                     start=False, stop=(c == NE - 1))
                for c in range(NE):
                    nc.tensor.matmul(psB,
                                     OHT[c][:, qt_i * 128:(qt_i + 1) * 128],
                                     WO[c][:, 512:768],
                                     start=False, stop=(c == NE - 1))
                fin = finp.tile([128, E], F32, name="fin", tag="fin")
                nc.scalar.copy(fin[:, 0:512], psA)
                nc.sync.dma_start(out=y[qt_i * 128:(qt_i + 1) * 128, 0:512],
                                  in_=fin[:, 0:512])
                nc.scalar.copy(fin[:, 512:768], psB)
                nc.sync.dma_start(out=y[qt_i * 128:(qt_i + 1) * 128, 512:768],
                                  in_=fin[:, 512:768])


_NC_CACHE = {}


def _get_nc():
    if "nc" not in _NC_CACHE:
        nc = bacc.Bacc("TRN2", target_bir_lowering=False, debug=False,
                       num_devices=8)
        with tile.TileContext(nc) as tc:
            _emit(tc)
        nc.compile()
        _NC_CACHE["nc"] = nc
    return _NC_CACHE["nc"]


def kernel(x, w_qkv, w_out, b_out):
    x = np.ascontiguousarray(x, dtype=np.float32)
    w_qkv = np.ascontiguousarray(w_qkv, dtype=np.float32)
    w_out = np.ascontiguousarray(w_out, dtype=np.float32)
    b_out = np.ascontiguousarray(b_out, dtype=np.float32)
    assert x.shape == (B, N, E)

    nc = _get_nc()
    in_maps = []
    for c in range(8):
        beta, qoff = c // 4, (c % 4) * NQ
        xbc = np.ascontiguousarray(np.roll(x[beta], -qoff, axis=0))
        in_maps.append({"xb": xbc, "wqkv": w_qkv, "wout": w_out,
                        "bout": b_out})
    res = bass_utils.run_bass_kernel_spmd(nc, in_maps, core_ids=list(range(8)))
    out = np.empty((B, N, E), dtype=np.float32)
    for c in range(8):
        beta, qoff = c // 4, (c % 4) * NQ
        out[beta, qoff:qoff + NQ, :] = res.results[c]["y"]
    return out
